# revision 16
# baseline (speedup 1.0000x reference)
"""MixtureOfAttention forward for Trainium2 (8 NeuronCores, data-parallel over B).

Math (exactly equivalent to the reference):
  s_t    = rsqrt(mean(x_t^2) + eps)                      (per token)
  logits = s * (x @ (norm_w ⊙ router_w)) + router_b
  r      = softmax(logits)                               [B, 4]
  y      = x + sum_e (r_e * s) * (x_e @ W_e) + r @ C
  W_e    = diag(norm_w_e) @ Wv_e @ proj_w_e @ out_w_e    [512, 2048] (host-folded)
  C_e    = proj_b_e @ out_w_e                            [2048]      (host-folded)
(seq_len==1 attention is the identity on v; r @ C is applied on host from the
device-computed routing probs and is exactly zero for proj_b == 0.)

Device strategy (per 128-token tile):
  - cast x*32 -> bf16, PE-transpose to feature-major xT16
  - router matmuls in bf16 (an fp8 router fails the error budget)
  - rmsnorm scale s via DVE square-reduce + scalar ln/exp (all scalar-engine
    functions {copy, ln, exp} live in one act table set -> 1 table load total)
  - fold r_e*s*M/(32*b_e) into a per-token-per-expert bf16 scaling of x, then
    transpose and quantize to fp8 on the PSUM->SBUF copyback; all experts can
    then accumulate into ONE psum group (combine = single pass y = x + z/M)
  - expert GEMMs in fp8(e4m3) DoubleRow mode (2 k-planes per instruction),
    with host-side error-feedback planes (W_COMP) for extra accuracy margin
"""

import os
import sys

sys.path.insert(0, "/opt/trn_rl_repo")

import numpy as np
import ml_dtypes

import concourse.bass as bass
import concourse.bacc as bacc
import concourse.mybir as mybir
import concourse.tile as tile
from concourse import bass_utils, masks

B, D, E = 32768, 2048, 4
dE = D // E  # 512
EPS = 1e-6
N_CORES = 8
P = 128
BC = B // N_CORES  # tokens per core
KC = D // P  # 16 feature chunks
EK = dE // P  # 4 chunks per expert
NB = 4  # psum banks for z (512 cols each)

_dt = mybir.dt
AF = mybir.ActivationFunctionType
ALU = mybir.AluOpType
DR = mybir.MatmulPerfMode.DoubleRow

A_SCALE = 32.0  # x pre-scale folded into cast1 (and divided out of Rw16)
M_SCALE = float(2 ** 15)  # fp8 q headroom; combine multiplies z by 1/M
W_COMP = os.environ.get("W_COMP", "0") == "1"  # host error-feedback planes
NV = 4 if W_COMP else 2  # DoubleRow plane-pairs per expert


def build(nt: int):
    """Build + compile the per-core kernel for nt tiles of 128 tokens."""
    bc = nt * P
    nc = bacc.Bacc("TRN2", target_bir_lowering=False, debug=False, num_devices=N_CORES)

    x_d = nc.dram_tensor("x", [bc, D], _dt.float32r, kind="ExternalInput")
    w_d = nc.dram_tensor("w8", [P, E, NV, 2, D], _dt.float8e4, kind="ExternalInput")
    rw_d = nc.dram_tensor("rw", [P, KC, E], _dt.bfloat16, kind="ExternalInput")
    rb_d = nc.dram_tensor("rb", [P, E], _dt.float32, kind="ExternalInput")
    cc_d = nc.dram_tensor("cconst", [P, E], _dt.float32, kind="ExternalInput")
    y_d = nc.dram_tensor("y", [bc, D], _dt.float32, kind="ExternalOutput")
    rt_d = nc.dram_tensor("routing", [bc, E], _dt.float32, kind="ExternalOutput")

    x_ap = x_d.ap()
    w_ap = w_d.ap()
    rw_ap = rw_d.ap()
    rb_ap = rb_d.ap()
    cc_ap = cc_d.ap()
    y_ap = y_d.ap()
    rt_ap = rt_d.ap()

    with tile.TileContext(nc) as tc:
        with (
            tc.tile_pool(name="const", bufs=1) as cpool,
            tc.tile_pool(name="xin", bufs=3) as xpool,
            tc.tile_pool(name="x16", bufs=2) as x16pool,
            tc.tile_pool(name="b16", bufs=2) as b16pool,
            tc.tile_pool(name="xt16", bufs=2) as xt16pool,
            tc.tile_pool(name="xt8", bufs=2) as xt8pool,
            tc.tile_pool(name="yout", bufs=2) as ypool,
            tc.tile_pool(name="sq", bufs=2) as sqpool,
            tc.tile_pool(name="small", bufs=4) as spool,
            tc.tile_pool(name="tp", bufs=3, space="PSUM") as tppool,
            tc.tile_pool(name="racc", bufs=1, space="PSUM") as rpool,
            tc.tile_pool(name="z", bufs=1, space="PSUM") as zpool,
        ):
            # ---- constants ----
            id32 = cpool.tile([P, P], _dt.float32, tag="id32")
            masks.make_identity(nc, id32[:])
            ident_r = cpool.tile([P, P], _dt.float32r, tag="identr")
            nc.vector.tensor_copy(ident_r[:], id32[:])
            eps_sb = cpool.tile([P, 1], _dt.float32, tag="eps")
            nc.vector.memset(eps_sb[:], float(EPS))
            ones32 = cpool.tile([P, P], _dt.float32, tag="ones32")
            nc.vector.memset(ones32[:], 1.0)
            ones16 = cpool.tile([P, P], _dt.bfloat16, tag="ones16")
            nc.vector.tensor_copy(ones16[:], ones32[:])

            # ---- PE warmup (p-state ramp); reuses the z psum ring ----
            warm = zpool.tile([P, NB, 512], _dt.float32, tag="z")
            for _ in range(40):
                nc.tensor.matmul(
                    warm[:, 0, 0:128],
                    ident_r[:],
                    ident_r[:],
                    start=True,
                    stop=True,
                )

            # ---- load weights (W dma interleaved with first x tiles) ----
            Rw_sb = cpool.tile([P, KC, E], _dt.bfloat16, tag="Rw")
            rb_sb = cpool.tile([P, E], _dt.float32, tag="rb")
            cc_sb = cpool.tile([P, E], _dt.float32, tag="cc")
            nc.sync.dma_start(Rw_sb[:], rw_ap)
            nc.sync.dma_start(rb_sb[:], rb_ap)
            nc.sync.dma_start(cc_sb[:], cc_ap)
            W_sb = cpool.tile([P, E, NV, 2, D], _dt.float8e4, tag="W8")
            prefetched = {}
            w_dmas = [(e, v) for e in range(E) for v in range(NV)]
            wi = 0
            for i in range(min(3, nt)):
                xs = xpool.tile([P, D], _dt.float32r, tag="x")
                nc.sync.dma_start(xs[:], x_ap[bass.ts(i, P), :])
                prefetched[i] = xs
                take = NV if i < 2 else len(w_dmas) - wi
                for e, v in w_dmas[wi : wi + take]:
                    nc.sync.dma_start(W_sb[:, e, v, :, :], w_ap[:, e, v, :, :])
                wi += take
            for e, v in w_dmas[wi:]:
                nc.sync.dma_start(W_sb[:, e, v, :, :], w_ap[:, e, v, :, :])

            def front_half(i):
                if i in prefetched:
                    xs = prefetched.pop(i)
                else:
                    xs = xpool.tile([P, D], _dt.float32r, tag="x")
                    nc.sync.dma_start(xs[:], x_ap[bass.ts(i, P), :])
                x32 = xs[:].bitcast(_dt.float32)

                # rms: ssq = mean(x^2) via scalar Square+accum (scale folds 1/D);
                # Square/Ln/Exp/Copy all share one act table set
                sq = sqpool.tile([P, D], _dt.float32, tag="sq")
                ssq = spool.tile([P, 1], _dt.float32, tag="ssq")
                nc.scalar.activation(
                    sq[:], x32, AF.Square, scale=float(D ** -0.5), accum_out=ssq[:]
                )
                lnm = spool.tile([P, 1], _dt.float32, tag="lnm")
                nc.scalar.activation(lnm[:], ssq[:], AF.Ln, bias=eps_sb[:])
                s_sb = spool.tile([P, 1], _dt.float32, tag="s")
                nc.scalar.activation(s_sb[:], lnm[:], AF.Exp, scale=-0.5)

                # PE: transpose x (fp32r) -> copyback casts to bf16 xT16
                xT16 = xt16pool.tile([P, KC, P], _dt.bfloat16, tag="xT16")
                for g in range(KC // 4):
                    tp = tppool.tile([P, 4, P], _dt.float32r, tag="tp")
                    for j4 in range(4):
                        k = 4 * g + j4
                        nc.tensor.transpose(
                            tp[:, j4, :],
                            xs[:, k * P : (k + 1) * P],
                            ident_r[:],
                        )
                    nc.vector.tensor_copy(
                        xT16[:, 4 * g : 4 * g + 4, :], tp[:].bitcast(_dt.float32)
                    )

                # PE: router logits in bf16
                racc = rpool.tile([P, E], _dt.float32, tag="racc")
                for k in range(KC):
                    nc.tensor.matmul(
                        racc[:],
                        xT16[:, k, :],
                        Rw_sb[:, k, :],
                        start=(k == 0),
                        stop=(k == KC - 1),
                    )

                logits = spool.tile([P, E], _dt.float32, tag="logits")
                nc.vector.scalar_tensor_tensor(
                    logits[:], racc[:], s_sb[:], rb_sb[:],
                    op0=ALU.mult, op1=ALU.add,
                )
                mx = spool.tile([P, 1], _dt.float32, tag="mx")
                nc.vector.reduce_max(mx[:], logits[:], axis=mybir.AxisListType.X)
                nm = spool.tile([P, 1], _dt.float32, tag="nm")
                nc.vector.tensor_scalar_mul(nm[:], mx[:], -1.0)
                exps = spool.tile([P, E], _dt.float32, tag="exps")
                se = spool.tile([P, 1], _dt.float32, tag="se")
                nc.scalar.activation(
                    exps[:], logits[:], AF.Exp, bias=nm[:], scale=1.0, accum_out=se[:]
                )
                rec = spool.tile([P, 1], _dt.float32, tag="rec")
                nc.vector.reciprocal(rec[:], se[:])
                routing = spool.tile([P, E], _dt.float32, tag="routing")
                nc.vector.tensor_scalar_mul(routing[:], exps[:], rec[:])
                nc.sync.dma_start(rt_ap[bass.ts(i, P), :], routing[:])
                rs = spool.tile([P, E], _dt.float32, tag="rs")
                nc.vector.tensor_scalar_mul(rs[:], routing[:], s_sb[:])
                coef = spool.tile([P, E], _dt.float32, tag="coef")
                nc.vector.tensor_mul(coef[:], rs[:], cc_sb[:])

                return xs, xT16, coef


            def back_half(i, xs, xT16, coef):
                x32 = xs[:].bitcast(_dt.float32)

                # broadcast coef to feature-major: coefT -> 4 bf16 outer
                # products (after the previous tile's experts in PE order,
                # so the PE never stalls on the softmax chain)
                tpc = tppool.tile([P, 4, P], _dt.float32r, tag="tp")
                nc.tensor.transpose(
                    tpc[0:E, 0, :].bitcast(_dt.float32), coef[:], id32[:]
                )
                ctr16 = spool.tile([E, P], _dt.bfloat16, tag="ctr16")
                nc.vector.tensor_copy(ctr16[:], tpc[0:E, 0, :].bitcast(_dt.float32))
                ctflat = spool.tile([1, E * P], _dt.bfloat16, tag="ctflat")
                nc.sync.dma_start(ctflat[:], ctr16[:])
                tpb = tppool.tile([P, 4, P], _dt.float32r, tag="tp")
                nc.tensor.matmul(
                    tpb[:].bitcast(_dt.float32),
                    ones16[0:1, :],
                    ctflat[:],
                    start=True,
                    stop=True,
                )
                B16 = b16pool.tile([P, E, P], _dt.bfloat16, tag="B16")
                nc.scalar.copy(B16[:], tpb[:].bitcast(_dt.float32))

                # scale+quantize: xT8 = fp8(xT16 * B), split DVE/gpsimd
                xT8 = xt8pool.tile([P, KC, P], _dt.float8e4, tag="xT8")
                for e in range(E):
                    eng = nc.vector if e < 2 else nc.gpsimd
                    eng.tensor_mul(
                        xT8[:, 4 * e : 4 * e + 4, :],
                        xT16[:, 4 * e : 4 * e + 4, :],
                        B16[:, e : e + 1, :].broadcast_to([P, 4, P]),
                    )
                # PE: expert GEMMs, all experts+planes into one psum group
                z = zpool.tile([P, NB, 512], _dt.float32, tag="z")
                ev = [(e, v) for e in range(E) for v in range(NV)]
                for idx, (e, v) in enumerate(ev):
                    lhsT = xT8[:, 4 * e + 2 * (v % 2) : 4 * e + 2 * (v % 2) + 2, :]
                    for j in range(NB):
                        nc.tensor.matmul(
                            z[:, j, :],
                            lhsT,
                            W_sb[:, e, v, :, bass.ts(j, 512)],
                            start=(idx == 0),
                            stop=(idx == len(ev) - 1),
                            perf_mode=DR,
                        )

                # combine y = x + z/M on DVE
                y = ypool.tile([P, D], _dt.float32, tag="y")
                for j in range(NB):
                    nc.vector.scalar_tensor_tensor(
                        y[:, bass.ts(j, 512)],
                        z[:, j, :],
                        1.0 / M_SCALE,
                        x32[:, bass.ts(j, 512)],
                        op0=ALU.mult,
                        op1=ALU.add,
                    )
                    nc.sync.dma_start(
                        y_ap[bass.ts(i, P), bass.ts(j, 512)], y[:, bass.ts(j, 512)]
                    )

            prev = None
            for i in range(nt):
                cur = front_half(i)
                if prev is not None:
                    back_half(i - 1, *prev)
                prev = cur
            back_half(nt - 1, *prev)

    nc.compile()
    return nc


LDW_OPT = os.environ.get("LDW_OPT", "0") == "1"


def _patch_ldw_opt():
    """Flip walrus --enable-ldw-opt so LDWEIGHTS overlap the previous matmul."""
    import concourse.bass_utils as bu

    if getattr(bu, "_ldw_patched", False):
        return
    orig = bu.run_command

    def patched(argv, **kwargs):
        if LDW_OPT:
            argv = [
                a.replace("--enable-ldw-opt=false", "--enable-ldw-opt=true")
                for a in argv
            ]
        return orig(argv, **kwargs)

    bu.run_command = patched
    bu._ldw_patched = True


def _pin_act_table():
    """Keep every scalar activation servable by ONE table set
    (natural_log_exp_and_others: copy/identity/exp/ln/square), so
    insert_act_table_loads emits a single load instead of flip-flopping
    (1283ns per reload on the Scalar engine)."""
    import concourse.hw_specs as hs

    if getattr(hs, "_act_pinned", False):
        return
    orig = hs.get_activation_tables
    PIN = {"copy", "identity", "exp", "ln", "square", "memset_zero", "abs"}

    def patched(module_arch):
        tables = orig(module_arch)
        import concourse.mybir as mb

        pin_funcs = set()
        for name in PIN:
            try:
                pin_funcs.add(mb.ActivationFunctionType.from_pwp(name))
            except Exception:
                pass
        out = {}
        for name, funcs in tables.items():
            if name == "natural_log_exp_and_others":
                out[name] = funcs
            else:
                out[name] = funcs - pin_funcs
        return out

    hs.get_activation_tables = patched
    bacc.get_activation_tables = patched
    hs._act_pinned = True


_patch_ldw_opt()
_pin_act_table()

_built = {}


def _get_nc(nt: int):
    if nt not in _built:
        _built[nt] = build(nt)
    return _built[nt]


def prepare_weights(norm_w, router_w, router_b, qkv_w, proj_w, proj_b, out_w):
    """Host-side fold of all linear stages into per-expert fp8 [512, 2048]."""
    E4 = ml_dtypes.float8_e4m3
    nw = norm_w.astype(np.float64)
    Wv = qkv_w[:, :, 2 * dE :].astype(np.float64)
    pw = proj_w.astype(np.float64)
    ow = out_w.astype(np.float64)
    W8 = np.empty((P, E, NV, 2, D), dtype=E4)
    b_e = np.empty((E,), np.float64)
    C = np.empty((E, D), dtype=np.float64)
    for e in range(E):
        nw_e = nw[e * dE : (e + 1) * dE]
        ow_e = ow[e * dE : (e + 1) * dE, :]
        We = (nw_e[:, None] * Wv[e]) @ pw[e] @ ow_e  # [512, 2048]
        C[e] = proj_b[e].astype(np.float64) @ ow_e
        b_e[e] = 2.0 ** np.floor(np.log2(224.0 / np.abs(We).max()))
        Wq = (We * b_e[e]).astype(np.float32).astype(E4)  # [512, 2048]
        planes = [Wq]
        if W_COMP:
            Rq = ((We * b_e[e]).astype(np.float32) - Wq.astype(np.float32)).astype(E4)
            planes.append(Rq)
        for pi, pl in enumerate(planes):
            arr = pl.reshape(EK, P, D)  # k-chunk, partition, col
            for v2 in range(2):
                for kp in range(2):
                    W8[:, e, 2 * pi + v2, kp, :] = arr[2 * v2 + kp]
    # router: fold norm_w, divide by A_SCALE (x is pre-scaled by 32)
    rw_fold = (nw[:, None] * router_w.astype(np.float64)).astype(
        ml_dtypes.bfloat16
    )
    rw_dev = np.ascontiguousarray(
        rw_fold.reshape(KC, P, E).transpose(1, 0, 2)
    )  # [P, KC, E]
    rb_dev = np.tile(router_b.astype(np.float32)[None, :], (P, 1))
    cconst = np.tile(
        (M_SCALE / b_e).astype(np.float32)[None, :], (P, 1)
    )  # [P, E]
    return W8, rw_dev, rb_dev, cconst, C


def _ensure_ntff_hook():
    """Make NTFF profiling work: antenv in the image lacks axon_hooks, so
    register an in-memory shim module and wire the ctypes hook from
    trn_agent_boot (on PYTHONPATH via /root/.axon_site)."""
    import types

    import antenv

    try:
        from antenv import axon_hooks
    except ImportError:
        mod = types.ModuleType("antenv.axon_hooks")
        mod._hook = None
        mod.set_axon_ntff_profile_hook = lambda h: setattr(mod, "_hook", h)
        mod.get_axon_ntff_profile_hook = lambda: mod._hook
        sys.modules["antenv.axon_hooks"] = mod
        antenv.axon_hooks = mod
        axon_hooks = mod

    if axon_hooks.get_axon_ntff_profile_hook() is None:
        if "/root/.axon_site" not in sys.path:
            sys.path.insert(0, "/root/.axon_site")
        from trn_agent_boot.trn_boot import _ntff_profile_via_ctypes

        h = _ntff_profile_via_ctypes("/opt/axon/libaxon_pjrt.so")
        if h is not None:
            axon_hooks.set_axon_ntff_profile_hook(h)


def kernel(x, norm_w, router_w, router_b, qkv_w, proj_w, proj_b, out_w, _trace=False):
    if _trace:
        try:
            _ensure_ntff_hook()
        except Exception as e:  # profiling is best-effort
            print("ntff hook setup failed:", e)
    x = np.ascontiguousarray(np.asarray(x, dtype=np.float32))
    W8, rw_dev, rb_dev, cconst, C = prepare_weights(
        np.asarray(norm_w),
        np.asarray(router_w),
        np.asarray(router_b),
        np.asarray(qkv_w),
        np.asarray(proj_w),
        np.asarray(proj_b),
        np.asarray(out_w),
    )
    nt = BC // P
    nc = _get_nc(nt)
    in_maps = []
    for c in range(N_CORES):
        in_maps.append(
            {
                "x": x[c * BC : (c + 1) * BC],
                "w8": W8,
                "rw": rw_dev,
                "rb": rb_dev,
                "cconst": cconst,
            }
        )
    res = bass_utils.run_bass_kernel_spmd(
        nc, in_maps, core_ids=list(range(N_CORES)), trace=_trace
    )
    y = np.concatenate([res.results[c]["y"] for c in range(N_CORES)], axis=0)
    if np.any(C != 0.0):
        routing = np.concatenate(
            [res.results[c]["routing"] for c in range(N_CORES)], axis=0
        )
        y = (y.astype(np.float64) + routing.astype(np.float64) @ C).astype(np.float32)
    if _trace:
        kernel._last_results = res
    return y


# revision 18
# speedup vs baseline: 1.5300x; 1.5300x over previous
"""MixtureOfAttention forward for Trainium2 (8 NeuronCores, data-parallel over B).

Math (exactly equivalent to the reference):
  s_t    = rsqrt(mean(x_t^2) + eps)                      (per token)
  logits = s * (x @ (norm_w ⊙ router_w)) + router_b
  r      = softmax(logits)                               [B, 4]
  y      = x + sum_e (r_e * s) * (x_e @ W_e) + r @ C
  W_e    = diag(norm_w_e) @ Wv_e @ proj_w_e @ out_w_e    [512, 2048] (host-folded)
  C_e    = proj_b_e @ out_w_e                            [2048]      (host-folded)
(seq_len==1 attention is the identity on v; r @ C is applied on host from the
device-computed routing probs and is exactly zero for proj_b == 0.)

Device strategy (per 128-token tile):
  - cast x*32 -> bf16, PE-transpose to feature-major xT16
  - router matmuls in bf16 (an fp8 router fails the error budget)
  - rmsnorm scale s via DVE square-reduce + scalar ln/exp (all scalar-engine
    functions {copy, ln, exp} live in one act table set -> 1 table load total)
  - fold r_e*s*M/(32*b_e) into a per-token-per-expert bf16 scaling of x, then
    transpose and quantize to fp8 on the PSUM->SBUF copyback; all experts can
    then accumulate into ONE psum group (combine = single pass y = x + z/M)
  - expert GEMMs in fp8(e4m3) DoubleRow mode (2 k-planes per instruction),
    with host-side error-feedback planes (W_COMP) for extra accuracy margin
"""

import os
import sys

sys.path.insert(0, "/opt/trn_rl_repo")

import numpy as np
import ml_dtypes

import concourse.bass as bass
import concourse.bacc as bacc
import concourse.mybir as mybir
import concourse.tile as tile
from concourse import bass_utils, masks

B, D, E = 32768, 2048, 4
dE = D // E  # 512
EPS = 1e-6
N_CORES = 8
P = 128
BC = B // N_CORES  # tokens per core
KC = D // P  # 16 feature chunks
EK = dE // P  # 4 chunks per expert
NB = 4  # psum banks for z (512 cols each)

_dt = mybir.dt
AF = mybir.ActivationFunctionType
ALU = mybir.AluOpType
DR = mybir.MatmulPerfMode.DoubleRow

A_SCALE = 32.0  # x pre-scale folded into cast1 (and divided out of Rw16)
M_SCALE = float(2 ** 15)  # fp8 q headroom; combine multiplies z by 1/M
W_COMP = os.environ.get("W_COMP", "0") == "1"  # host error-feedback planes
NV = 4 if W_COMP else 2  # DoubleRow plane-pairs per expert


def build(nt: int):
    """Build + compile the per-core kernel for nt tiles of 128 tokens."""
    bc = nt * P
    nc = bacc.Bacc("TRN2", target_bir_lowering=False, debug=False, num_devices=N_CORES)

    x_d = nc.dram_tensor("x", [bc, D], _dt.float32r, kind="ExternalInput")
    w_d = nc.dram_tensor("w8", [P, E, NV, 2, D], _dt.float8e4, kind="ExternalInput")
    rw_d = nc.dram_tensor("rw", [P, KC, E], _dt.bfloat16, kind="ExternalInput")
    rb_d = nc.dram_tensor("rb", [P, E], _dt.float32, kind="ExternalInput")
    cc_d = nc.dram_tensor("cconst", [P, E], _dt.float32, kind="ExternalInput")
    y_d = nc.dram_tensor("y", [bc, D], _dt.float32, kind="ExternalOutput")
    rt_d = nc.dram_tensor("routing", [bc, E], _dt.float32, kind="ExternalOutput")

    x_ap = x_d.ap()
    w_ap = w_d.ap()
    rw_ap = rw_d.ap()
    rb_ap = rb_d.ap()
    cc_ap = cc_d.ap()
    y_ap = y_d.ap()
    rt_ap = rt_d.ap()

    with tile.TileContext(nc) as tc:
        with (
            tc.tile_pool(name="const", bufs=1) as cpool,
            tc.tile_pool(name="xin", bufs=6) as xpool,
            tc.tile_pool(name="x16", bufs=2) as x16pool,
            tc.tile_pool(name="b16", bufs=2) as b16pool,
            tc.tile_pool(name="xt16", bufs=4) as xt16pool,
            tc.tile_pool(name="xt8", bufs=2) as xt8pool,
            tc.tile_pool(name="yout", bufs=2) as ypool,
            tc.tile_pool(name="sq", bufs=2) as sqpool,
            tc.tile_pool(name="small", bufs=4) as spool,
            tc.tile_pool(name="tp", bufs=3, space="PSUM") as tppool,
            tc.tile_pool(name="racc", bufs=1, space="PSUM") as rpool,
            tc.tile_pool(name="z", bufs=1, space="PSUM") as zpool,
        ):
            # ---- constants ----
            id32 = cpool.tile([P, P], _dt.float32, tag="id32")
            masks.make_identity(nc, id32[:])
            ident_r = cpool.tile([P, P], _dt.float32r, tag="identr")
            nc.vector.tensor_copy(ident_r[:], id32[:])
            eps_sb = cpool.tile([P, 1], _dt.float32, tag="eps")
            nc.vector.memset(eps_sb[:], float(EPS))
            ones32 = cpool.tile([P, P], _dt.float32, tag="ones32")
            nc.vector.memset(ones32[:], 1.0)
            ones16 = cpool.tile([P, P], _dt.bfloat16, tag="ones16")
            nc.vector.tensor_copy(ones16[:], ones32[:])

            # ---- PE warmup (p-state ramp); reuses the z psum ring ----
            warm = zpool.tile([P, NB, 512], _dt.float32, tag="z")
            for _ in range(40):
                nc.tensor.matmul(
                    warm[:, 0, 0:128],
                    ident_r[:],
                    ident_r[:],
                    start=True,
                    stop=True,
                )

            # ---- load weights (W dma interleaved with first x tiles) ----
            Rw_sb = cpool.tile([P, KC, E], _dt.bfloat16, tag="Rw")
            rb_sb = cpool.tile([P, E], _dt.float32, tag="rb")
            cc_sb = cpool.tile([P, E], _dt.float32, tag="cc")
            nc.sync.dma_start(Rw_sb[:], rw_ap)
            nc.sync.dma_start(rb_sb[:], rb_ap)
            nc.sync.dma_start(cc_sb[:], cc_ap)
            W_sb = cpool.tile([P, E, NV, 2, D], _dt.float8e4, tag="W8")
            prefetched = {}
            w_dmas = [(e, v) for e in range(E) for v in range(NV)]
            wi = 0
            for i in range(min(3, nt)):
                xs = xpool.tile([P, D], _dt.float32r, tag="x")
                nc.sync.dma_start(xs[:], x_ap[bass.ts(i, P), :])
                prefetched[i] = xs
                take = NV if i < 2 else len(w_dmas) - wi
                for e, v in w_dmas[wi : wi + take]:
                    nc.sync.dma_start(W_sb[:, e, v, :, :], w_ap[:, e, v, :, :])
                wi += take
            for e, v in w_dmas[wi:]:
                nc.sync.dma_start(W_sb[:, e, v, :, :], w_ap[:, e, v, :, :])

            def front_half(i):
                if i in prefetched:
                    xs = prefetched.pop(i)
                else:
                    xs = xpool.tile([P, D], _dt.float32r, tag="x")
                    nc.sync.dma_start(xs[:], x_ap[bass.ts(i, P), :])
                x32 = xs[:].bitcast(_dt.float32)

                # rms: ssq = mean(x^2) via scalar Square+accum (scale folds 1/D);
                # Square/Ln/Exp/Copy all share one act table set
                sq = sqpool.tile([P, D], _dt.float32, tag="sq")
                ssq = spool.tile([P, 1], _dt.float32, tag="ssq")
                nc.scalar.activation(
                    sq[:], x32, AF.Square, scale=float(D ** -0.5), accum_out=ssq[:]
                )
                lnm = spool.tile([P, 1], _dt.float32, tag="lnm")
                nc.scalar.activation(lnm[:], ssq[:], AF.Ln, bias=eps_sb[:])
                s_sb = spool.tile([P, 1], _dt.float32, tag="s")
                nc.scalar.activation(s_sb[:], lnm[:], AF.Exp, scale=-0.5)

                # PE: transpose x (fp32r) -> copyback casts to bf16 xT16
                xT16 = xt16pool.tile([P, KC, P], _dt.bfloat16, tag="xT16")
                for g in range(KC // 4):
                    tp = tppool.tile([P, 4, P], _dt.float32r, tag="tp")
                    for j4 in range(4):
                        k = 4 * g + j4
                        nc.tensor.transpose(
                            tp[:, j4, :],
                            xs[:, k * P : (k + 1) * P],
                            ident_r[:],
                        )
                    nc.vector.tensor_copy(
                        xT16[:, 4 * g : 4 * g + 4, :], tp[:].bitcast(_dt.float32)
                    )

                # PE: router logits in bf16
                racc = rpool.tile([P, E], _dt.float32, tag="racc")
                for k in range(KC):
                    nc.tensor.matmul(
                        racc[:],
                        xT16[:, k, :],
                        Rw_sb[:, k, :],
                        start=(k == 0),
                        stop=(k == KC - 1),
                    )

                logits = spool.tile([P, E], _dt.float32, tag="logits")
                nc.vector.scalar_tensor_tensor(
                    logits[:], racc[:], s_sb[:], rb_sb[:],
                    op0=ALU.mult, op1=ALU.add,
                )
                mx = spool.tile([P, 1], _dt.float32, tag="mx")
                nc.vector.reduce_max(mx[:], logits[:], axis=mybir.AxisListType.X)
                nm = spool.tile([P, 1], _dt.float32, tag="nm")
                nc.vector.tensor_scalar_mul(nm[:], mx[:], -1.0)
                exps = spool.tile([P, E], _dt.float32, tag="exps")
                se = spool.tile([P, 1], _dt.float32, tag="se")
                nc.scalar.activation(
                    exps[:], logits[:], AF.Exp, bias=nm[:], scale=1.0, accum_out=se[:]
                )
                rec = spool.tile([P, 1], _dt.float32, tag="rec")
                nc.vector.reciprocal(rec[:], se[:])
                routing = spool.tile([P, E], _dt.float32, tag="routing")
                nc.vector.tensor_scalar_mul(routing[:], exps[:], rec[:])
                nc.sync.dma_start(rt_ap[bass.ts(i, P), :], routing[:])
                rs = spool.tile([P, E], _dt.float32, tag="rs")
                nc.vector.tensor_scalar_mul(rs[:], routing[:], s_sb[:])
                coef = spool.tile([P, E], _dt.float32, tag="coef")
                nc.vector.tensor_mul(coef[:], rs[:], cc_sb[:])

                return xs, xT16, coef


            def bcast_prep(i, coef):
                # coefT via PE transpose (dep: coef, one iteration old)
                tpc = tppool.tile([P, 4, P], _dt.float32r, tag="tp")
                nc.tensor.transpose(
                    tpc[0:E, 0, :].bitcast(_dt.float32), coef[:], id32[:]
                )
                ctr16 = spool.tile([E, P], _dt.bfloat16, tag="ctr16")
                nc.vector.tensor_copy(ctr16[:], tpc[0:E, 0, :].bitcast(_dt.float32))
                ctflat = spool.tile([1, E * P], _dt.bfloat16, tag="ctflat")
                nc.sync.dma_start(ctflat[:], ctr16[:])
                return ctflat

            def outer_stage(i, xT16, ctflat):
                # one K=1 outer product -> B[feat, (e, tok)] (dep: ctflat,
                # one iteration old)
                tpb = tppool.tile([P, 4, P], _dt.float32r, tag="tp")
                nc.tensor.matmul(
                    tpb[:].bitcast(_dt.float32),
                    ones16[0:1, :],
                    ctflat[:],
                    start=True,
                    stop=True,
                )
                B16 = b16pool.tile([P, E, P], _dt.bfloat16, tag="B16")
                nc.scalar.copy(B16[:], tpb[:].bitcast(_dt.float32))
                xT8 = xt8pool.tile([P, KC, P], _dt.float8e4, tag="xT8")
                for e in range(E):
                    eng = nc.vector if e < 2 else nc.gpsimd
                    eng.tensor_mul(
                        xT8[:, 4 * e : 4 * e + 4, :],
                        xT16[:, 4 * e : 4 * e + 4, :],
                        B16[:, e : e + 1, :].broadcast_to([P, 4, P]),
                    )
                return xT8

            def tail(i, xs, xT8):
                x32 = xs[:].bitcast(_dt.float32)
                # PE: expert GEMMs, all experts+planes into one psum group
                z = zpool.tile([P, NB, 512], _dt.float32, tag="z")
                ev = [(e, v) for e in range(E) for v in range(NV)]
                for idx, (e, v) in enumerate(ev):
                    lhsT = xT8[:, 4 * e + 2 * (v % 2) : 4 * e + 2 * (v % 2) + 2, :]
                    for j in range(NB):
                        nc.tensor.matmul(
                            z[:, j, :],
                            lhsT,
                            W_sb[:, e, v, :, bass.ts(j, 512)],
                            start=(idx == 0),
                            stop=(idx == len(ev) - 1),
                            perf_mode=DR,
                        )

                # combine y = x + z/M on DVE
                y = ypool.tile([P, D], _dt.float32, tag="y")
                for j in range(NB):
                    nc.vector.scalar_tensor_tensor(
                        y[:, bass.ts(j, 512)],
                        z[:, j, :],
                        1.0 / M_SCALE,
                        x32[:, bass.ts(j, 512)],
                        op0=ALU.mult,
                        op1=ALU.add,
                    )
                    nc.sync.dma_start(
                        y_ap[bass.ts(i, P), bass.ts(j, 512)], y[:, bass.ts(j, 512)]
                    )

            # 4-deep software pipeline: every PE instruction's deps are
            # at least one full iteration old, so the in-order PE queue
            # never blocks on the softmax/broadcast engine chains.
            #   iter i: front(i) | bcast_prep(i-1) | outer(i-2) | tail(i-3)
            F = {}   # i -> (xs, xT16, coef)
            CF = {}  # i -> ctflat
            X8 = {}  # i -> xT8
            for i in range(nt + 3):
                if i < nt:
                    F[i] = front_half(i)
                if 0 <= i - 1 < nt:
                    CF[i - 1] = bcast_prep(i - 1, F[i - 1][2])
                if 0 <= i - 2 < nt:
                    X8[i - 2] = outer_stage(i - 2, F[i - 2][1], CF.pop(i - 2))
                if 0 <= i - 3 < nt:
                    tail(i - 3, F[i - 3][0], X8.pop(i - 3))
                    del F[i - 3]
    nc.compile()
    return nc


LDW_OPT = os.environ.get("LDW_OPT", "0") == "1"


def _patch_ldw_opt():
    """Flip walrus --enable-ldw-opt so LDWEIGHTS overlap the previous matmul."""
    import concourse.bass_utils as bu

    if getattr(bu, "_ldw_patched", False):
        return
    orig = bu.run_command

    def patched(argv, **kwargs):
        if LDW_OPT:
            argv = [
                a.replace("--enable-ldw-opt=false", "--enable-ldw-opt=true")
                for a in argv
            ]
        return orig(argv, **kwargs)

    bu.run_command = patched
    bu._ldw_patched = True


def _pin_act_table():
    """Keep every scalar activation servable by ONE table set
    (natural_log_exp_and_others: copy/identity/exp/ln/square), so
    insert_act_table_loads emits a single load instead of flip-flopping
    (1283ns per reload on the Scalar engine)."""
    import concourse.hw_specs as hs

    if getattr(hs, "_act_pinned", False):
        return
    orig = hs.get_activation_tables
    PIN = {"copy", "identity", "exp", "ln", "square", "memset_zero", "abs"}

    def patched(module_arch):
        tables = orig(module_arch)
        import concourse.mybir as mb

        pin_funcs = set()
        for name in PIN:
            try:
                pin_funcs.add(mb.ActivationFunctionType.from_pwp(name))
            except Exception:
                pass
        out = {}
        for name, funcs in tables.items():
            if name == "natural_log_exp_and_others":
                out[name] = funcs
            else:
                out[name] = funcs - pin_funcs
        return out

    hs.get_activation_tables = patched
    bacc.get_activation_tables = patched
    hs._act_pinned = True


_patch_ldw_opt()
_pin_act_table()

_built = {}


def _get_nc(nt: int):
    if nt not in _built:
        _built[nt] = build(nt)
    return _built[nt]


def prepare_weights(norm_w, router_w, router_b, qkv_w, proj_w, proj_b, out_w):
    """Host-side fold of all linear stages into per-expert fp8 [512, 2048]."""
    E4 = ml_dtypes.float8_e4m3
    nw = norm_w.astype(np.float64)
    Wv = qkv_w[:, :, 2 * dE :].astype(np.float64)
    pw = proj_w.astype(np.float64)
    ow = out_w.astype(np.float64)
    W8 = np.empty((P, E, NV, 2, D), dtype=E4)
    b_e = np.empty((E,), np.float64)
    C = np.empty((E, D), dtype=np.float64)
    for e in range(E):
        nw_e = nw[e * dE : (e + 1) * dE]
        ow_e = ow[e * dE : (e + 1) * dE, :]
        We = (nw_e[:, None] * Wv[e]) @ pw[e] @ ow_e  # [512, 2048]
        C[e] = proj_b[e].astype(np.float64) @ ow_e
        b_e[e] = 2.0 ** np.floor(np.log2(224.0 / np.abs(We).max()))
        Wq = (We * b_e[e]).astype(np.float32).astype(E4)  # [512, 2048]
        planes = [Wq]
        if W_COMP:
            Rq = ((We * b_e[e]).astype(np.float32) - Wq.astype(np.float32)).astype(E4)
            planes.append(Rq)
        for pi, pl in enumerate(planes):
            arr = pl.reshape(EK, P, D)  # k-chunk, partition, col
            for v2 in range(2):
                for kp in range(2):
                    W8[:, e, 2 * pi + v2, kp, :] = arr[2 * v2 + kp]
    # router: fold norm_w, divide by A_SCALE (x is pre-scaled by 32)
    rw_fold = (nw[:, None] * router_w.astype(np.float64)).astype(
        ml_dtypes.bfloat16
    )
    rw_dev = np.ascontiguousarray(
        rw_fold.reshape(KC, P, E).transpose(1, 0, 2)
    )  # [P, KC, E]
    rb_dev = np.tile(router_b.astype(np.float32)[None, :], (P, 1))
    cconst = np.tile(
        (M_SCALE / b_e).astype(np.float32)[None, :], (P, 1)
    )  # [P, E]
    return W8, rw_dev, rb_dev, cconst, C


def _ensure_ntff_hook():
    """Make NTFF profiling work: antenv in the image lacks axon_hooks, so
    register an in-memory shim module and wire the ctypes hook from
    trn_agent_boot (on PYTHONPATH via /root/.axon_site)."""
    import types

    import antenv

    try:
        from antenv import axon_hooks
    except ImportError:
        mod = types.ModuleType("antenv.axon_hooks")
        mod._hook = None
        mod.set_axon_ntff_profile_hook = lambda h: setattr(mod, "_hook", h)
        mod.get_axon_ntff_profile_hook = lambda: mod._hook
        sys.modules["antenv.axon_hooks"] = mod
        antenv.axon_hooks = mod
        axon_hooks = mod

    if axon_hooks.get_axon_ntff_profile_hook() is None:
        if "/root/.axon_site" not in sys.path:
            sys.path.insert(0, "/root/.axon_site")
        from trn_agent_boot.trn_boot import _ntff_profile_via_ctypes

        h = _ntff_profile_via_ctypes("/opt/axon/libaxon_pjrt.so")
        if h is not None:
            axon_hooks.set_axon_ntff_profile_hook(h)


def kernel(x, norm_w, router_w, router_b, qkv_w, proj_w, proj_b, out_w, _trace=False):
    if _trace:
        try:
            _ensure_ntff_hook()
        except Exception as e:  # profiling is best-effort
            print("ntff hook setup failed:", e)
    x = np.ascontiguousarray(np.asarray(x, dtype=np.float32))
    W8, rw_dev, rb_dev, cconst, C = prepare_weights(
        np.asarray(norm_w),
        np.asarray(router_w),
        np.asarray(router_b),
        np.asarray(qkv_w),
        np.asarray(proj_w),
        np.asarray(proj_b),
        np.asarray(out_w),
    )
    nt = BC // P
    nc = _get_nc(nt)
    in_maps = []
    for c in range(N_CORES):
        in_maps.append(
            {
                "x": x[c * BC : (c + 1) * BC],
                "w8": W8,
                "rw": rw_dev,
                "rb": rb_dev,
                "cconst": cconst,
            }
        )
    res = bass_utils.run_bass_kernel_spmd(
        nc, in_maps, core_ids=list(range(N_CORES)), trace=_trace
    )
    y = np.concatenate([res.results[c]["y"] for c in range(N_CORES)], axis=0)
    if np.any(C != 0.0):
        routing = np.concatenate(
            [res.results[c]["routing"] for c in range(N_CORES)], axis=0
        )
        y = (y.astype(np.float64) + routing.astype(np.float64) @ C).astype(np.float32)
    if _trace:
        kernel._last_results = res
    return y


# revision 20
# speedup vs baseline: 1.5506x; 1.0135x over previous
"""MixtureOfAttention forward for Trainium2 (8 NeuronCores, data-parallel over B).

Math (exactly equivalent to the reference):
  s_t    = rsqrt(mean(x_t^2) + eps)                      (per token)
  logits = s * (x @ (norm_w ⊙ router_w)) + router_b
  r      = softmax(logits)                               [B, 4]
  y      = x + sum_e (r_e * s) * (x_e @ W_e) + r @ C
  W_e    = diag(norm_w_e) @ Wv_e @ proj_w_e @ out_w_e    [512, 2048] (host-folded)
  C_e    = proj_b_e @ out_w_e                            [2048]      (host-folded)
(seq_len==1 attention is the identity on v; r @ C is applied on host from the
device-computed routing probs and is exactly zero for proj_b == 0.)

Device strategy (per 128-token tile):
  - cast x*32 -> bf16, PE-transpose to feature-major xT16
  - router matmuls in bf16 (an fp8 router fails the error budget)
  - rmsnorm scale s via DVE square-reduce + scalar ln/exp (all scalar-engine
    functions {copy, ln, exp} live in one act table set -> 1 table load total)
  - fold r_e*s*M/(32*b_e) into a per-token-per-expert bf16 scaling of x, then
    transpose and quantize to fp8 on the PSUM->SBUF copyback; all experts can
    then accumulate into ONE psum group (combine = single pass y = x + z/M)
  - expert GEMMs in fp8(e4m3) DoubleRow mode (2 k-planes per instruction),
    with host-side error-feedback planes (W_COMP) for extra accuracy margin
"""

import os
import sys

sys.path.insert(0, "/opt/trn_rl_repo")

import numpy as np
import ml_dtypes

import concourse.bass as bass
import concourse.bacc as bacc
import concourse.mybir as mybir
import concourse.tile as tile
from concourse import bass_utils, masks

B, D, E = 32768, 2048, 4
dE = D // E  # 512
EPS = 1e-6
N_CORES = 8
P = 128
BC = B // N_CORES  # tokens per core
KC = D // P  # 16 feature chunks
EK = dE // P  # 4 chunks per expert
NB = 4  # psum banks for z (512 cols each)

_dt = mybir.dt
AF = mybir.ActivationFunctionType
ALU = mybir.AluOpType
DR = mybir.MatmulPerfMode.DoubleRow

A_SCALE = 32.0  # x pre-scale folded into cast1 (and divided out of Rw16)
M_SCALE = float(2 ** 15)  # fp8 q headroom; combine multiplies z by 1/M
W_COMP = os.environ.get("W_COMP", "0") == "1"  # host error-feedback planes
NV = 4 if W_COMP else 2  # DoubleRow plane-pairs per expert


def build(nt: int):
    """Build + compile the per-core kernel for nt tiles of 128 tokens."""
    bc = nt * P
    nc = bacc.Bacc("TRN2", target_bir_lowering=False, debug=False, num_devices=N_CORES)

    x_d = nc.dram_tensor("x", [bc, D], _dt.float32r, kind="ExternalInput")
    w_d = nc.dram_tensor("w8", [P, E, NV, 2, D], _dt.float8e4, kind="ExternalInput")
    rw_d = nc.dram_tensor("rw", [P, KC, E], _dt.bfloat16, kind="ExternalInput")
    rb_d = nc.dram_tensor("rb", [P, E], _dt.float32, kind="ExternalInput")
    cc_d = nc.dram_tensor("cconst", [P, E], _dt.float32, kind="ExternalInput")
    y_d = nc.dram_tensor("y", [bc, D], _dt.float32, kind="ExternalOutput")
    rt_d = nc.dram_tensor("routing", [bc, E], _dt.float32, kind="ExternalOutput")

    x_ap = x_d.ap()
    w_ap = w_d.ap()
    y_ap = y_d.ap()
    rt_ap = rt_d.ap()

    with tile.TileContext(nc) as tc:
        with (
            tc.tile_pool(name="const", bufs=1) as cpool,
            tc.tile_pool(name="xin", bufs=8) as xpool,
            tc.tile_pool(name="x16", bufs=3) as x16pool,
            tc.tile_pool(name="xt16", bufs=4) as xt16pool,
            tc.tile_pool(name="xt8", bufs=2) as xt8pool,
            tc.tile_pool(name="yout", bufs=2) as ypool,
            tc.tile_pool(name="sq", bufs=2) as sqpool,
            tc.tile_pool(name="b16", bufs=2) as b16pool,
            tc.tile_pool(name="small", bufs=5) as spool,
            tc.tile_pool(name="tp", bufs=3, space="PSUM") as tppool,
            tc.tile_pool(name="racc", bufs=1, space="PSUM") as rpool,
            tc.tile_pool(name="z", bufs=1, space="PSUM") as zpool,
        ):
            # ---- constants ----
            id32 = cpool.tile([P, P], _dt.float32, tag="id32")
            masks.make_identity(nc, id32[:])
            id16 = cpool.tile([P, P], _dt.bfloat16, tag="id16")
            nc.vector.tensor_copy(id16[:], id32[:])
            eps_sb = cpool.tile([P, 1], _dt.float32, tag="eps")
            nc.vector.memset(eps_sb[:], float(EPS))
            ones16 = cpool.tile([P, P], _dt.bfloat16, tag="ones16")
            nc.vector.memset(ones16[:], 1.0)

            # ---- PE warmup (p-state ramp); reuses the z psum ring ----
            warm = zpool.tile([P, NB, 512], _dt.float32, tag="z")
            for _ in range(40):
                nc.tensor.matmul(
                    warm[:, 0, 0:128], id16[:], id16[:], start=True, stop=True
                )

            # ---- load weights + router consts ----
            Rw_sb = cpool.tile([P, KC, E], _dt.bfloat16, tag="Rw")
            rb_sb = cpool.tile([P, E], _dt.float32, tag="rb")
            cc_sb = cpool.tile([P, E], _dt.float32, tag="cc")
            nc.sync.dma_start(Rw_sb[:], rw_d.ap())
            nc.sync.dma_start(rb_sb[:], rb_d.ap())
            nc.sync.dma_start(cc_sb[:], cc_d.ap())
            W_sb = cpool.tile([P, E, NV, 2, D], _dt.float8e4, tag="W8")
            for e in range(E):
                for v in range(NV):
                    nc.sync.dma_start(W_sb[:, e, v, :, :], w_ap[:, e, v, :, :])

            XS = {}

            def fetch(j):
                """DMA x tile j (issued >=3 iterations before use)."""
                xs = xpool.tile([P, D], _dt.float32r, tag="x")
                nc.sync.dma_start(xs[:], x_ap[bass.ts(j, P), :])
                XS[j] = xs

            def cast_stage(j):
                """bf16 cast + rmsnorm scale for tile j (one iter early)."""
                xs = XS[j]
                x32 = xs[:].bitcast(_dt.float32)
                x16 = x16pool.tile([P, D], _dt.bfloat16, tag="x16")
                nc.scalar.activation(x16[:], x32, AF.Copy, scale=1.0)
                sq = sqpool.tile([P, D], _dt.float32, tag="sq")
                ssq = spool.tile([P, 1], _dt.float32, tag="ssq")
                nc.scalar.activation(
                    sq[:], x32, AF.Square, scale=float(D ** -0.5), accum_out=ssq[:]
                )
                lnm = spool.tile([P, 1], _dt.float32, tag="lnm")
                nc.scalar.activation(lnm[:], ssq[:], AF.Ln, bias=eps_sb[:])
                s_sb = spool.tile([P, 1], _dt.float32, tag="s")
                nc.scalar.activation(s_sb[:], lnm[:], AF.Exp, scale=-0.5)
                return x16, s_sb

            def front_half(i, x16, s_sb):
                """PE transposes + router + softmax -> coef for tile i."""
                xT16 = xt16pool.tile([P, KC, P], _dt.bfloat16, tag="xT16")
                for g in range(KC // 4):
                    tp = tppool.tile([P, 4, P], _dt.float32r, tag="tp")
                    tp16 = tp[:, :, 0:64].bitcast(_dt.bfloat16)
                    for j4 in range(4):
                        k = 4 * g + j4
                        nc.tensor.transpose(
                            tp16[:, j4, :], x16[:, k * P : (k + 1) * P], id16[:]
                        )
                    nc.vector.tensor_copy(xT16[:, 4 * g : 4 * g + 4, :], tp16[:])

                racc = rpool.tile([P, E], _dt.float32, tag="racc")
                for k in range(KC):
                    nc.tensor.matmul(
                        racc[:],
                        xT16[:, k, :],
                        Rw_sb[:, k, :],
                        start=(k == 0),
                        stop=(k == KC - 1),
                    )

                logits = spool.tile([P, E], _dt.float32, tag="logits")
                nc.vector.scalar_tensor_tensor(
                    logits[:], racc[:], s_sb[:], rb_sb[:],
                    op0=ALU.mult, op1=ALU.add,
                )
                mx = spool.tile([P, 1], _dt.float32, tag="mx")
                nc.vector.reduce_max(mx[:], logits[:], axis=mybir.AxisListType.X)
                nm = spool.tile([P, 1], _dt.float32, tag="nm")
                nc.vector.tensor_scalar_mul(nm[:], mx[:], -1.0)
                exps = spool.tile([P, E], _dt.float32, tag="exps")
                se = spool.tile([P, 1], _dt.float32, tag="se")
                nc.scalar.activation(
                    exps[:], logits[:], AF.Exp, bias=nm[:], scale=1.0, accum_out=se[:]
                )
                rec = spool.tile([P, 1], _dt.float32, tag="rec")
                nc.vector.reciprocal(rec[:], se[:])
                routing = spool.tile([P, E], _dt.float32, tag="routing")
                nc.vector.tensor_scalar_mul(routing[:], exps[:], rec[:])
                nc.sync.dma_start(rt_ap[bass.ts(i, P), :], routing[:])
                rs = spool.tile([P, E], _dt.float32, tag="rs")
                nc.vector.tensor_scalar_mul(rs[:], routing[:], s_sb[:])
                coef = spool.tile([P, E], _dt.float32, tag="coef")
                nc.vector.tensor_mul(coef[:], rs[:], cc_sb[:])
                return xT16, coef

            def bcast_prep(i, coef):
                """coefT via PE transpose + flatten-DMA (deps one iter old)."""
                tpc = tppool.tile([P, 4, P], _dt.float32r, tag="tp")
                nc.tensor.transpose(
                    tpc[0:E, 0, :].bitcast(_dt.float32), coef[:], id32[:]
                )
                ctr16 = spool.tile([E, P], _dt.bfloat16, tag="ctr16")
                nc.vector.tensor_copy(ctr16[:], tpc[0:E, 0, :].bitcast(_dt.float32))
                ctflat = spool.tile([1, E * P], _dt.bfloat16, tag="ctflat")
                nc.scalar.dma_start(ctflat[:], ctr16[:])
                return ctflat

            def outer_pe(i, ctflat):
                """K=1 outer product -> B[feat,(e,tok)] + bf16 copy (scalar)."""
                tpb = tppool.tile([P, 4, P], _dt.float32r, tag="tp")
                nc.tensor.matmul(
                    tpb[:].bitcast(_dt.float32),
                    ones16[0:1, :],
                    ctflat[:],
                    start=True,
                    stop=True,
                )
                B16 = b16pool.tile([P, E, P], _dt.bfloat16, tag="B16")
                nc.scalar.copy(B16[:], tpb[:].bitcast(_dt.float32))
                return B16

            def mults(i, xT16, B16):
                xT8 = xt8pool.tile([P, KC, P], _dt.float8e4, tag="xT8")
                for e in range(E):
                    eng = nc.vector if e < 2 else nc.gpsimd
                    eng.tensor_mul(
                        xT8[:, 4 * e : 4 * e + 4, :],
                        xT16[:, 4 * e : 4 * e + 4, :],
                        B16[:, e : e + 1, :].broadcast_to([P, 4, P]),
                    )
                return xT8

            def tail(i, xs, xT8):
                x32 = xs[:].bitcast(_dt.float32)
                z = zpool.tile([P, NB, 512], _dt.float32, tag="z")
                ev = [(e, v) for e in range(E) for v in range(NV)]
                for idx, (e, v) in enumerate(ev):
                    lhsT = xT8[:, 4 * e + 2 * (v % 2) : 4 * e + 2 * (v % 2) + 2, :]
                    for j in range(NB):
                        nc.tensor.matmul(
                            z[:, j, :],
                            lhsT,
                            W_sb[:, e, v, :, bass.ts(j, 512)],
                            start=(idx == 0),
                            stop=(idx == len(ev) - 1),
                            perf_mode=DR,
                        )
                y = ypool.tile([P, D], _dt.float32, tag="y")
                for j in range(NB):
                    nc.vector.scalar_tensor_tensor(
                        y[:, bass.ts(j, 512)],
                        z[:, j, :],
                        1.0 / M_SCALE,
                        x32[:, bass.ts(j, 512)],
                        op0=ALU.mult,
                        op1=ALU.add,
                    )
                    nc.sync.dma_start(
                        y_ap[bass.ts(i, P), bass.ts(j, 512)], y[:, bass.ts(j, 512)]
                    )

            # 4-deep software pipeline; x DMA runs 3 iterations ahead and the
            # bf16 cast + rmsnorm one iteration ahead, so no PE instruction
            # ever waits on a same-iteration engine or DMA chain.
            for j in range(min(3, nt)):
                fetch(j)
            C16 = {}  # j -> (x16, s)
            F = {}    # i -> (xs, xT16, coef)
            CF = {}   # i -> ctflat
            TPB = {}  # i -> outer psum tile
            X8 = {}   # i -> xT8
            if nt > 0:
                C16[0] = cast_stage(0)
            for i in range(nt + 3):
                if 0 <= i - 2 < nt and (i - 2) in CF:
                    TPB[i - 2] = outer_pe(i - 2, CF.pop(i - 2))
                if i < nt:
                    x16, s_sb = C16.pop(i)
                    xT16, coef = front_half(i, x16, s_sb)
                    F[i] = (XS.pop(i), xT16, coef)
                if 0 <= i - 2 < nt and (i - 2) in TPB:
                    X8[i - 2] = mults(i - 2, F[i - 2][1], TPB.pop(i - 2))
                if 0 <= i - 1 < nt:
                    CF[i - 1] = bcast_prep(i - 1, F[i - 1][2])
                if i + 3 < nt:
                    fetch(i + 3)
                if i + 1 < nt:
                    C16[i + 1] = cast_stage(i + 1)
                if 0 <= i - 3 < nt:
                    tail(i - 3, F[i - 3][0], X8.pop(i - 3))
                    del F[i - 3]

    nc.compile()
    return nc


LDW_OPT = os.environ.get("LDW_OPT", "0") == "1"


def _patch_ldw_opt():
    """Flip walrus --enable-ldw-opt so LDWEIGHTS overlap the previous matmul."""
    import concourse.bass_utils as bu

    if getattr(bu, "_ldw_patched", False):
        return
    orig = bu.run_command

    def patched(argv, **kwargs):
        if LDW_OPT:
            argv = [
                a.replace("--enable-ldw-opt=false", "--enable-ldw-opt=true")
                for a in argv
            ]
        return orig(argv, **kwargs)

    bu.run_command = patched
    bu._ldw_patched = True


def _pin_act_table():
    """Keep every scalar activation servable by ONE table set
    (natural_log_exp_and_others: copy/identity/exp/ln/square), so
    insert_act_table_loads emits a single load instead of flip-flopping
    (1283ns per reload on the Scalar engine)."""
    import concourse.hw_specs as hs

    if getattr(hs, "_act_pinned", False):
        return
    orig = hs.get_activation_tables
    PIN = {"copy", "identity", "exp", "ln", "square", "memset_zero", "abs"}

    def patched(module_arch):
        tables = orig(module_arch)
        import concourse.mybir as mb

        pin_funcs = set()
        for name in PIN:
            try:
                pin_funcs.add(mb.ActivationFunctionType.from_pwp(name))
            except Exception:
                pass
        out = {}
        for name, funcs in tables.items():
            if name == "natural_log_exp_and_others":
                out[name] = funcs
            else:
                out[name] = funcs - pin_funcs
        return out

    hs.get_activation_tables = patched
    bacc.get_activation_tables = patched
    hs._act_pinned = True


_patch_ldw_opt()
_pin_act_table()

_built = {}


def _get_nc(nt: int):
    if nt not in _built:
        _built[nt] = build(nt)
    return _built[nt]


def prepare_weights(norm_w, router_w, router_b, qkv_w, proj_w, proj_b, out_w):
    """Host-side fold of all linear stages into per-expert fp8 [512, 2048]."""
    E4 = ml_dtypes.float8_e4m3
    nw = norm_w.astype(np.float64)
    Wv = qkv_w[:, :, 2 * dE :].astype(np.float64)
    pw = proj_w.astype(np.float64)
    ow = out_w.astype(np.float64)
    W8 = np.empty((P, E, NV, 2, D), dtype=E4)
    b_e = np.empty((E,), np.float64)
    C = np.empty((E, D), dtype=np.float64)
    for e in range(E):
        nw_e = nw[e * dE : (e + 1) * dE]
        ow_e = ow[e * dE : (e + 1) * dE, :]
        We = (nw_e[:, None] * Wv[e]) @ pw[e] @ ow_e  # [512, 2048]
        C[e] = proj_b[e].astype(np.float64) @ ow_e
        b_e[e] = 2.0 ** np.floor(np.log2(224.0 / np.abs(We).max()))
        Wq = (We * b_e[e]).astype(np.float32).astype(E4)  # [512, 2048]
        planes = [Wq]
        if W_COMP:
            Rq = ((We * b_e[e]).astype(np.float32) - Wq.astype(np.float32)).astype(E4)
            planes.append(Rq)
        for pi, pl in enumerate(planes):
            arr = pl.reshape(EK, P, D)  # k-chunk, partition, col
            for v2 in range(2):
                for kp in range(2):
                    W8[:, e, 2 * pi + v2, kp, :] = arr[2 * v2 + kp]
    # router: fold norm_w, divide by A_SCALE (x is pre-scaled by 32)
    rw_fold = (nw[:, None] * router_w.astype(np.float64)).astype(
        ml_dtypes.bfloat16
    )
    rw_dev = np.ascontiguousarray(
        rw_fold.reshape(KC, P, E).transpose(1, 0, 2)
    )  # [P, KC, E]
    rb_dev = np.tile(router_b.astype(np.float32)[None, :], (P, 1))
    cconst = np.tile(
        (M_SCALE / b_e).astype(np.float32)[None, :], (P, 1)
    )  # [P, E]
    return W8, rw_dev, rb_dev, cconst, C


def _ensure_ntff_hook():
    """Make NTFF profiling work: antenv in the image lacks axon_hooks, so
    register an in-memory shim module and wire the ctypes hook from
    trn_agent_boot (on PYTHONPATH via /root/.axon_site)."""
    import types

    import antenv

    try:
        from antenv import axon_hooks
    except ImportError:
        mod = types.ModuleType("antenv.axon_hooks")
        mod._hook = None
        mod.set_axon_ntff_profile_hook = lambda h: setattr(mod, "_hook", h)
        mod.get_axon_ntff_profile_hook = lambda: mod._hook
        sys.modules["antenv.axon_hooks"] = mod
        antenv.axon_hooks = mod
        axon_hooks = mod

    if axon_hooks.get_axon_ntff_profile_hook() is None:
        if "/root/.axon_site" not in sys.path:
            sys.path.insert(0, "/root/.axon_site")
        from trn_agent_boot.trn_boot import _ntff_profile_via_ctypes

        h = _ntff_profile_via_ctypes("/opt/axon/libaxon_pjrt.so")
        if h is not None:
            axon_hooks.set_axon_ntff_profile_hook(h)


def kernel(x, norm_w, router_w, router_b, qkv_w, proj_w, proj_b, out_w, _trace=False):
    if _trace:
        try:
            _ensure_ntff_hook()
        except Exception as e:  # profiling is best-effort
            print("ntff hook setup failed:", e)
    x = np.ascontiguousarray(np.asarray(x, dtype=np.float32))
    W8, rw_dev, rb_dev, cconst, C = prepare_weights(
        np.asarray(norm_w),
        np.asarray(router_w),
        np.asarray(router_b),
        np.asarray(qkv_w),
        np.asarray(proj_w),
        np.asarray(proj_b),
        np.asarray(out_w),
    )
    nt = BC // P
    nc = _get_nc(nt)
    in_maps = []
    for c in range(N_CORES):
        in_maps.append(
            {
                "x": x[c * BC : (c + 1) * BC],
                "w8": W8,
                "rw": rw_dev,
                "rb": rb_dev,
                "cconst": cconst,
            }
        )
    res = bass_utils.run_bass_kernel_spmd(
        nc, in_maps, core_ids=list(range(N_CORES)), trace=_trace
    )
    y = np.concatenate([res.results[c]["y"] for c in range(N_CORES)], axis=0)
    if np.any(C != 0.0):
        routing = np.concatenate(
            [res.results[c]["routing"] for c in range(N_CORES)], axis=0
        )
        y = (y.astype(np.float64) + routing.astype(np.float64) @ C).astype(np.float32)
    if _trace:
        kernel._last_results = res
    return y
